# revision 5
# baseline (speedup 1.0000x reference)
"""Trainium2 Bass kernel for nn_AttentionModule (dense transformer block).

Computation (per batch element b):
    q = X @ Wq.T ; k = K @ Wk.T ; v = X @ Wv.T        (X=query_input, K=key_input)
    a = softmax((k @ q.T) / sqrt(D), axis=-1)          -> (NK, NQ)
    out = a @ v + K                                    -> (NK, D)

Sharding: data-parallel over batch, one batch element per NeuronCore (B == 8).

q and k never appear individually -- only the Gram product k @ q.T does.
Folding the two projection weights into G = Wq.T @ Wk (host-side weight
pre-pack) turns the score matrix into S.T = X @ (G @ K.T), which removes the
entire q projection from the device: device work drops from 15.0 to 12.9
GMAC per core.

All matmuls run in fp8e4m3 with DoubleRow perf mode (2 contraction rows per
cell per cycle), accumulating in fp32 PSUM.  Every operand lives in a single
3D SBUF tile [128, n_sub, cols] where dim 1 indexes contiguous 128-row
blocks of the contraction axis; a DoubleRow matmul consumes a
[:, 2k:2k+2, :] slice (256-row contraction step).  This layout lets each
DRAM tensor load with one or two large rearranged DMA descriptors (the
~1 us per-descriptor DGE overhead otherwise dominates the fill), and fp8
shrinks kg (= G @ K.T) and v to 2 MB each so both stay SBUF-resident
between phases -- no DRAM spill round-trips.  PSUM evacuation copies
alternate between the vector and scalar engines so neither gates the PE.
exp runs on the scalar engine writing fp8 directly; the softmax denominator
is a ones-vector DoubleRow matmul over the same fp8 exp tiles (numerator /
denominator quantization errors partially cancel), and the normalization is
folded into the output pass as a fused per-partition multiply-add on the
vector engine in fp32.
"""

import numpy as np
import ml_dtypes

import concourse.tile as tile
from concourse import bacc, mybir
from concourse.bass_utils import run_bass_kernel_spmd
from concourse.masks import make_identity

B, NQ, NK, D = 8, 2048, 2048, 1024
P = 128
DSB = D // P         # 8 feature sub-blocks
NSB = NQ // P        # 16 query sub-blocks
DBP = DSB // 2       # 4 DoubleRow steps over features
NBP = NSB // 2       # 8 DoubleRow steps over queries
NB = NQ // P         # 16 query-row blocks
MC = 512             # scores chunk width (n_k columns per chunk)
NMC = NK // MC       # 4 chunks
SCALE = 1.0 / float(np.sqrt(np.float32(D)))

F32 = mybir.dt.float32
F8 = mybir.dt.float8e4
DR = mybir.MatmulPerfMode.DoubleRow

_CACHE = {}


def _sub(ap):
    """[R, C] dram AP -> [128, R//128, C] with dim 1 = contiguous row blocks."""
    return ap.rearrange("(s p) n -> p s n", p=P)


def _build():
    nc = bacc.Bacc("TRN2", target_bir_lowering=False, debug=False, num_devices=B)

    x8 = nc.dram_tensor("x8", [D, NQ], F8, kind="ExternalInput").ap()
    kt8 = nc.dram_tensor("kt8", [D, NK], F8, kind="ExternalInput").ap()
    knat = nc.dram_tensor("knat", [NK, D], F32, kind="ExternalInput").ap()
    g8 = nc.dram_tensor("g8", [D, D], F8, kind="ExternalInput").ap()
    wv8 = nc.dram_tensor("wv8", [D, D], F8, kind="ExternalInput").ap()
    out = nc.dram_tensor("out", [NK, D], F32, kind="ExternalOutput").ap()

    with tile.TileContext(nc) as tc:
        with (
            tc.tile_pool(name="const", bufs=1) as constp,
            tc.tile_pool(name="big", bufs=1) as bigp,
            tc.tile_pool(name="psum", bufs=1, space="PSUM") as psp,
        ):
            ident = constp.tile([1, 1], F32, tag="ident", name="ident")
            make_identity(nc, ident)
            # DoubleRow stationary APs need dim-1 stride % 16 == 0, so the
            # ones vector is padded to 16 columns (output rows identical;
            # row 0 is consumed)
            ones = constp.tile([P, 2, 16], F8, tag="ones", name="ones")
            nc.vector.memset(ones, 1.0)

            # input operands as per-column-block 3D tiles, one large
            # rearranged DMA each (Tile's dependency tracking is per-tile
            # coarse-interval, so a consumer of one tile never waits on
            # another block's load); kgall/vall are filled from PSUM by
            # phase 1 and consumed by phase 2 (SBUF-resident, no DRAM spill)
            g_h = [bigp.tile([P, DSB, D // 2], F8, tag=f"g{h}", name="gh")
                   for h in range(2)]
            kt_q = [bigp.tile([P, DSB, NK // 4], F8, tag=f"kt{q}", name="ktq")
                    for q in range(4)]
            x_q = [bigp.tile([P, DSB, NQ // 4], F8, tag=f"x{q}", name="xq")
                   for q in range(4)]
            wvall = bigp.tile([P, DSB, D], F8, tag="wvall", name="wvall")
            kgall = bigp.tile([P, DSB, NK], F8, tag="kgall", name="kgall")
            vall = bigp.tile([P, NSB, D], F8, tag="vall", name="vall")

            # loads in first-consumed order; the kt quarters alternate
            # between the gpsimd and sync DGE rings so the stream keeps up
            # with phase-1a consumption, g/wv lead on sync, x follows on
            # both rings for phase 1b
            nc.sync.dma_start(out=g_h[0], in_=_sub(g8[:, 0:D // 2]))
            nc.gpsimd.dma_start(out=kt_q[0], in_=_sub(kt8[:, 0:512]))
            nc.sync.dma_start(out=g_h[1], in_=_sub(g8[:, D // 2:D]))
            nc.sync.dma_start(out=kt_q[1], in_=_sub(kt8[:, 512:1024]))
            nc.gpsimd.dma_start(out=kt_q[2], in_=_sub(kt8[:, 1024:1536]))
            nc.sync.dma_start(out=kt_q[3], in_=_sub(kt8[:, 1536:2048]))
            nc.gpsimd.dma_start(out=x_q[0], in_=_sub(x8[:, 0:512]))
            nc.sync.dma_start(out=wvall, in_=_sub(wv8))
            nc.gpsimd.dma_start(out=x_q[1], in_=_sub(x8[:, 512:1024]))
            nc.gpsimd.dma_start(out=x_q[2], in_=_sub(x8[:, 1024:1536]))
            nc.gpsimd.dma_start(out=x_q[3], in_=_sub(x8[:, 1536:2048]))

            # ---------------- phase 1: projections ----------------
            # -- kg.T[d, m] = sum_e gT[e, d] * K.T[e, m]
            # (gT = G.T = Wk.T @ Wq supplied by host; output row-block db
            #  lands in kgall[:, db, :])
            gi = 0
            for mc4 in range(NK // 512):
                for db in range(DSB):
                    tg = "mm" if gi % 2 == 0 else "st2"
                    ps = psp.tile([P, 512], F32, tag=tg, name="mm", bufs=2)
                    for ebp in range(DBP):
                        nc.tensor.matmul(
                            ps,
                            g_h[db // 4][:, 2 * ebp:2 * ebp + 2,
                                         (db % 4) * P:(db % 4 + 1) * P],
                            kt_q[mc4][:, 2 * ebp:2 * ebp + 2, :],
                            start=(ebp == 0),
                            stop=(ebp == DBP - 1),
                            perf_mode=DR,
                        )
                    dst = kgall[:, db, mc4 * 512:(mc4 + 1) * 512]
                    if gi % 2 == 0:
                        nc.vector.tensor_copy(dst, ps)
                    else:
                        nc.scalar.copy(dst, ps)
                    gi += 1

            # -- v[n, dv] = sum_d X.T[d, n] * Wv.T[d, dv]
            # (output row-block nb lands in vall[:, nb, :])
            for nb in range(NB):
                for dc in range(D // 512):
                    gi += 1
                    tg = "mm" if gi % 2 == 0 else "st2"
                    ps = psp.tile([P, 512], F32, tag=tg, name="mm", bufs=2)
                    for dbp in range(DBP):
                        nc.tensor.matmul(
                            ps,
                            x_q[nb // 4][:, 2 * dbp:2 * dbp + 2,
                                         (nb % 4) * P:(nb % 4 + 1) * P],
                            wvall[:, 2 * dbp:2 * dbp + 2, dc * 512:(dc + 1) * 512],
                            start=(dbp == 0),
                            stop=(dbp == DBP - 1),
                            perf_mode=DR,
                        )
                    dst = vall[:, nb, dc * 512:(dc + 1) * 512]
                    if gi % 2 == 0:
                        nc.vector.tensor_copy(dst, ps)
                    else:
                        nc.scalar.copy(dst, ps)

            # ---------------- phase 2: attention ----------------
            with (
                tc.tile_pool(name="expst", bufs=2) as expp,
                tc.tile_pool(name="knp", bufs=2) as knp,
                tc.tile_pool(name="outp", bufs=6) as outp,
                tc.tile_pool(name="small", bufs=4) as smallp,
            ):
                for mc in range(NMC):
                    m0 = mc * MC

                    # scores + exp + column-sum accumulation
                    # exp of row-block nb lands in expst[:, nb, :]
                    expst = expp.tile([P, NSB, MC], F8, tag="expst", name="expst")
                    cs_ps = psp.tile([16, MC], F32, tag="csrp", name="cs", bufs=2)
                    for jp in range(NBP):
                        st_ps = psp.tile([P, 2 * MC], F32, tag="st2", name="st",
                                         bufs=2)
                        for half in range(2):
                            nb = 2 * jp + half
                            for dbp in range(DBP):
                                nc.tensor.matmul(
                                    st_ps[:, half * MC:(half + 1) * MC],
                                    x_q[nb // 4][:, 2 * dbp:2 * dbp + 2,
                                                 (nb % 4) * P:(nb % 4 + 1) * P],
                                    kgall[:, 2 * dbp:2 * dbp + 2, m0:m0 + MC],
                                    start=(dbp == 0),
                                    stop=(dbp == DBP - 1),
                                    perf_mode=DR,
                                )
                        # one exp per pair over the 2-bank PSUM tile
                        nc.scalar.activation(
                            out=expst[:, 2 * jp:2 * jp + 2, :], in_=st_ps,
                            func=mybir.ActivationFunctionType.Exp, scale=SCALE,
                        )
                        # the column-sum matmul for pair j is emitted one
                        # pair late so the exp -> cs semaphore never gates PE
                        if jp >= 1:
                            j = jp - 1
                            nc.tensor.matmul(
                                cs_ps, ones, expst[:, 2 * j:2 * j + 2, :],
                                start=(j == 0), stop=False, perf_mode=DR,
                            )
                    nc.tensor.matmul(
                        cs_ps, ones, expst[:, NSB - 2:NSB, :],
                        start=False, stop=True, perf_mode=DR,
                    )
                    # residual rows for this chunk: one big rearranged load,
                    # emitted late so it never delays the input fill on the
                    # serial DMA engine
                    knt = knp.tile([P, 4, D], F32, tag="knat", name="knat")
                    nc.sync.dma_start(out=knt, in_=_sub(knat[m0:m0 + MC, :]))

                    # context: C[m, dv] = sum_n expst[n, m] * v[n, dv]
                    # The first group's matmuls are emitted BEFORE the
                    # reciprocal/transpose chain so the PE fills the
                    # reciprocal's DVE latency with context work (only the
                    # stt consumer needs recip_pp).
                    def ctx_group(msb, dc):
                        c_ps = psp.tile([P, 512], F32, tag="mm", name="mm", bufs=2)
                        for nbp in range(NBP):
                            nc.tensor.matmul(
                                c_ps,
                                expst[:, 2 * nbp:2 * nbp + 2,
                                      msb * P:(msb + 1) * P],
                                vall[:, 2 * nbp:2 * nbp + 2,
                                     dc * 512:(dc + 1) * 512],
                                start=(nbp == 0),
                                stop=(nbp == NBP - 1),
                                perf_mode=DR,
                            )
                        return c_ps

                    ot0 = outp.tile([P, D], F32, tag="ostage", name="ostage")
                    c_ps00 = ctx_group(0, 0)

                    recip_row = smallp.tile([1, MC], F32, tag="rrow", name="rrow")
                    nc.vector.reciprocal(recip_row, cs_ps[0:1, :])
                    rp_ps = psp.tile([P, MC // P], F32, tag="csrp", name="rp", bufs=2)
                    for j in range(MC // P):
                        nc.tensor.transpose(
                            rp_ps[:, j:j + 1],
                            recip_row[:, j * P:(j + 1) * P],
                            ident,
                        )
                    recip_pp = smallp.tile([P, MC // P], F32, tag="rpp", name="rpp")
                    nc.vector.tensor_copy(recip_pp, rp_ps)

                    for msb in range(MC // P):
                        r0 = m0 + msb * P
                        ot = ot0 if msb == 0 else outp.tile(
                            [P, D], F32, tag="ostage", name="ostage")
                        for dc in range(D // 512):
                            c_ps = c_ps00 if (msb == 0 and dc == 0) \
                                else ctx_group(msb, dc)
                            nc.vector.scalar_tensor_tensor(
                                out=ot[:, dc * 512:(dc + 1) * 512],
                                in0=c_ps,
                                scalar=recip_pp[:, msb:msb + 1],
                                in1=knt[:, msb, dc * 512:(dc + 1) * 512],
                                op0=mybir.AluOpType.mult,
                                op1=mybir.AluOpType.add,
                            )
                        nc.scalar.dma_start(out=out[r0:r0 + P, :], in_=ot)

    nc.compile()
    return nc


def _get_nc():
    if "nc" not in _CACHE:
        _CACHE["nc"] = _build()
    return _CACHE["nc"]


def _prep_in_maps(query_input, key_input, Wq, Wk, Wv):
    f8 = ml_dtypes.float8_e4m3
    query_input = np.asarray(query_input, dtype=np.float32)
    key_input = np.asarray(key_input, dtype=np.float32)
    Wq = np.asarray(Wq, dtype=np.float32)
    Wk = np.asarray(Wk, dtype=np.float32)
    Wv = np.asarray(Wv, dtype=np.float32)
    # weight pre-pack: g8 = G.T = (Wq.T @ Wk).T = Wk.T @ Wq, so that
    # kg.T = g8.T @ K.T on device with g8 blocks as the stationary operand
    g8 = np.ascontiguousarray(Wk.T @ Wq).astype(f8)
    wv8 = np.ascontiguousarray(Wv.T).astype(f8)
    in_maps = []
    for b in range(B):
        in_maps.append({
            "x8": np.ascontiguousarray(query_input[b].T).astype(f8),
            "kt8": np.ascontiguousarray(key_input[b].T).astype(f8),
            "knat": np.ascontiguousarray(key_input[b]),
            "g8": g8,
            "wv8": wv8,
        })
    return in_maps


def kernel(query_input, key_input, Wq, Wk, Wv):
    nc = _get_nc()
    in_maps = _prep_in_maps(query_input, key_input, Wq, Wk, Wv)
    res = run_bass_kernel_spmd(nc, in_maps, list(range(B))).results
    return np.stack([res[b]["out"] for b in range(B)], axis=0)


# revision 6
# speedup vs baseline: 1.0989x; 1.0989x over previous
"""Trainium2 Bass kernel for nn_AttentionModule (dense transformer block).

Computation (per batch element b):
    q = X @ Wq.T ; k = K @ Wk.T ; v = X @ Wv.T        (X=query_input, K=key_input)
    a = softmax((k @ q.T) / sqrt(D), axis=-1)          -> (NK, NQ)
    out = a @ v + K                                    -> (NK, D)

Sharding: data-parallel over batch, one batch element per NeuronCore (B == 8).

q and k never appear individually -- only the Gram product k @ q.T does.
Folding the two projection weights into G = Wq.T @ Wk (host-side weight
pre-pack) turns the score matrix into S.T = X @ (G @ K.T), which removes the
entire q projection from the device: device work drops from 15.0 to 12.9
GMAC per core.

All matmuls run in fp8e4m3 with DoubleRow perf mode (2 contraction rows per
cell per cycle), accumulating in fp32 PSUM.  Every operand lives in a single
3D SBUF tile [128, n_sub, cols] where dim 1 indexes contiguous 128-row
blocks of the contraction axis; a DoubleRow matmul consumes a
[:, 2k:2k+2, :] slice (256-row contraction step).  This layout lets each
DRAM tensor load with one or two large rearranged DMA descriptors (the
~1 us per-descriptor DGE overhead otherwise dominates the fill), and fp8
shrinks kg (= G @ K.T) and v to 2 MB each so both stay SBUF-resident
between phases -- no DRAM spill round-trips.  PSUM evacuation copies
alternate between the vector and scalar engines so neither gates the PE.
exp runs on the scalar engine writing fp8 directly; the softmax denominator
is a ones-vector DoubleRow matmul over the same fp8 exp tiles (numerator /
denominator quantization errors partially cancel), and the normalization is
folded into the output pass as a fused per-partition multiply-add on the
vector engine in fp32.
"""

import numpy as np
import ml_dtypes

import concourse.tile as tile
from concourse import bacc, mybir
from concourse.bass_utils import run_bass_kernel_spmd
from concourse.masks import make_identity

B, NQ, NK, D = 8, 2048, 2048, 1024
P = 128
DSB = D // P         # 8 feature sub-blocks
NSB = NQ // P        # 16 query sub-blocks
DBP = DSB // 2       # 4 DoubleRow steps over features
NBP = NSB // 2       # 8 DoubleRow steps over queries
NB = NQ // P         # 16 query-row blocks
MC = 512             # scores chunk width (n_k columns per chunk)
NMC = NK // MC       # 4 chunks
SCALE = 1.0 / float(np.sqrt(np.float32(D)))

F32 = mybir.dt.float32
F8 = mybir.dt.float8e4
DR = mybir.MatmulPerfMode.DoubleRow

_CACHE = {}


def _sub(ap):
    """[R, C] dram AP -> [128, R//128, C] with dim 1 = contiguous row blocks."""
    return ap.rearrange("(s p) n -> p s n", p=P)


def _build():
    nc = bacc.Bacc("TRN2", target_bir_lowering=False, debug=False, num_devices=B)

    x8 = nc.dram_tensor("x8", [D, NQ], F8, kind="ExternalInput").ap()
    kt8 = nc.dram_tensor("kt8", [D, NK], F8, kind="ExternalInput").ap()
    knat = nc.dram_tensor("knat", [NK, D], F32, kind="ExternalInput").ap()
    g8 = nc.dram_tensor("g8", [D, D], F8, kind="ExternalInput").ap()
    wv8 = nc.dram_tensor("wv8", [D, D], F8, kind="ExternalInput").ap()
    out = nc.dram_tensor("out", [NK, D], F32, kind="ExternalOutput").ap()

    with tile.TileContext(nc) as tc:
        with (
            tc.tile_pool(name="const", bufs=1) as constp,
            tc.tile_pool(name="big", bufs=1) as bigp,
            tc.tile_pool(name="psum", bufs=1, space="PSUM") as psp,
        ):
            ident = constp.tile([1, 1], F32, tag="ident", name="ident")
            make_identity(nc, ident)
            # DoubleRow stationary APs need dim-1 stride % 16 == 0, so the
            # ones vector is padded to 16 columns (output rows identical;
            # row 0 is consumed)
            ones = constp.tile([P, 2, 16], F8, tag="ones", name="ones")
            nc.vector.memset(ones, 1.0)

            # input operands as per-column-block 3D tiles, one large
            # rearranged DMA each (Tile's dependency tracking is per-tile
            # coarse-interval, so a consumer of one tile never waits on
            # another block's load); kgall/vall are filled from PSUM by
            # phase 1 and consumed by phase 2 (SBUF-resident, no DRAM spill)
            g_h = [bigp.tile([P, DSB, D // 2], F8, tag=f"g{h}", name="gh")
                   for h in range(2)]
            kt_q = [bigp.tile([P, DSB, NK // 4], F8, tag=f"kt{q}", name="ktq")
                    for q in range(4)]
            x_q = [bigp.tile([P, DSB, NQ // 4], F8, tag=f"x{q}", name="xq")
                   for q in range(4)]
            wvall = bigp.tile([P, DSB, D], F8, tag="wvall", name="wvall")
            # per-chunk kg tiles: exact deps, so chunk-0 scores can be
            # scheduled as soon as its 8 projection groups are evacuated
            kg_c = [bigp.tile([P, DSB, MC], F8, tag=f"kg{q}", name="kgc")
                    for q in range(NMC)]
            vall = bigp.tile([P, NSB, D], F8, tag="vall", name="vall")

            # loads in first-consumed order; the kt quarters alternate
            # between the gpsimd and sync DGE rings so the stream keeps up
            # with phase-1a consumption, g/wv lead on sync, x follows on
            # both rings for phase 1b
            nc.gpsimd.dma_start(out=kt_q[0], in_=_sub(kt8[:, 0:512]))
            nc.sync.dma_start(out=g_h[0], in_=_sub(g8[:, 0:D // 2]))
            nc.sync.dma_start(out=g_h[1], in_=_sub(g8[:, D // 2:D]))
            nc.sync.dma_start(out=kt_q[1], in_=_sub(kt8[:, 512:1024]))
            nc.gpsimd.dma_start(out=kt_q[2], in_=_sub(kt8[:, 1024:1536]))
            nc.sync.dma_start(out=kt_q[3], in_=_sub(kt8[:, 1536:2048]))
            nc.gpsimd.dma_start(out=x_q[0], in_=_sub(x8[:, 0:512]))
            nc.sync.dma_start(out=wvall, in_=_sub(wv8))
            nc.gpsimd.dma_start(out=x_q[1], in_=_sub(x8[:, 512:1024]))
            nc.gpsimd.dma_start(out=x_q[2], in_=_sub(x8[:, 1024:1536]))
            nc.gpsimd.dma_start(out=x_q[3], in_=_sub(x8[:, 1536:2048]))

            # ---------------- phase 1: projections ----------------
            # -- kg.T[d, m] = sum_e gT[e, d] * K.T[e, m]
            # (gT = G.T = Wk.T @ Wq supplied by host; output row-block db
            #  lands in kgall[:, db, :])
            gi = 0
            for mc4 in range(NK // 512):
                for db in range(DSB):
                    tg = "mm" if gi % 2 == 0 else "st2"
                    ps = psp.tile([P, 512], F32, tag=tg, name="mm", bufs=2)
                    for ebp in range(DBP):
                        nc.tensor.matmul(
                            ps,
                            g_h[db // 4][:, 2 * ebp:2 * ebp + 2,
                                         (db % 4) * P:(db % 4 + 1) * P],
                            kt_q[mc4][:, 2 * ebp:2 * ebp + 2, :],
                            start=(ebp == 0),
                            stop=(ebp == DBP - 1),
                            perf_mode=DR,
                        )
                    dst = kg_c[mc4][:, db, :]
                    if gi % 2 == 0:
                        nc.vector.tensor_copy(dst, ps)
                    else:
                        nc.scalar.copy(dst, ps)
                    gi += 1

            # -- v[n, dv] = sum_d X.T[d, n] * Wv.T[d, dv]
            # (output row-block nb lands in vall[:, nb, :])
            for nb in range(NB):
                for dc in range(D // 512):
                    gi += 1
                    tg = "mm" if gi % 2 == 0 else "st2"
                    ps = psp.tile([P, 512], F32, tag=tg, name="mm", bufs=2)
                    for dbp in range(DBP):
                        nc.tensor.matmul(
                            ps,
                            x_q[nb // 4][:, 2 * dbp:2 * dbp + 2,
                                         (nb % 4) * P:(nb % 4 + 1) * P],
                            wvall[:, 2 * dbp:2 * dbp + 2, dc * 512:(dc + 1) * 512],
                            start=(dbp == 0),
                            stop=(dbp == DBP - 1),
                            perf_mode=DR,
                        )
                    dst = vall[:, nb, dc * 512:(dc + 1) * 512]
                    if gi % 2 == 0:
                        nc.vector.tensor_copy(dst, ps)
                    else:
                        nc.scalar.copy(dst, ps)

            # ---------------- phase 2: attention ----------------
            # Chunk-level software pipeline: scores(mc+1) is emitted after
            # ctx(mc), so every chunk's exp chain has the full context-phase
            # duration of slack before its own ctx consumes it.  Chunk 0's
            # scores run right after the projections.
            with (
                tc.tile_pool(name="expst", bufs=2) as expp,
                tc.tile_pool(name="knp", bufs=2) as knp,
                tc.tile_pool(name="outp", bufs=6) as outp,
                tc.tile_pool(name="small", bufs=4) as smallp,
            ):
                def emit_scores(mc):
                    # scores + exp + column-sum accumulation for chunk mc;
                    # exp of row-block nb lands in expst[:, nb, :]
                    expst = expp.tile([P, NSB, MC], F8, tag="expst", name="expst")
                    cs_ps = psp.tile([16, MC], F32, tag="csrp", name="cs", bufs=2)
                    for jp in range(NBP):
                        st_ps = psp.tile([P, 2 * MC], F32, tag="st2", name="st",
                                         bufs=2)
                        for half in range(2):
                            nb = 2 * jp + half
                            for dbp in range(DBP):
                                nc.tensor.matmul(
                                    st_ps[:, half * MC:(half + 1) * MC],
                                    x_q[nb // 4][:, 2 * dbp:2 * dbp + 2,
                                                 (nb % 4) * P:(nb % 4 + 1) * P],
                                    kg_c[mc][:, 2 * dbp:2 * dbp + 2, :],
                                    start=(dbp == 0),
                                    stop=(dbp == DBP - 1),
                                    perf_mode=DR,
                                )
                        # one exp per pair over the 2-bank PSUM tile
                        nc.scalar.activation(
                            out=expst[:, 2 * jp:2 * jp + 2, :], in_=st_ps,
                            func=mybir.ActivationFunctionType.Exp, scale=SCALE,
                        )
                        # the column-sum matmul for pair j is emitted one
                        # pair late so the exp -> cs semaphore never gates PE
                        if jp >= 1:
                            j = jp - 1
                            nc.tensor.matmul(
                                cs_ps, ones, expst[:, 2 * j:2 * j + 2, :],
                                start=(j == 0), stop=False, perf_mode=DR,
                            )
                    nc.tensor.matmul(
                        cs_ps, ones, expst[:, NSB - 2:NSB, :],
                        start=False, stop=True, perf_mode=DR,
                    )
                    return expst, cs_ps

                def emit_ctx(mc, expst, cs_ps):
                    m0 = mc * MC
                    # residual rows for this chunk: one big rearranged load,
                    # emitted late so it never delays the input fill on the
                    # serial DMA engine
                    knt = knp.tile([P, 4, D], F32, tag="knat", name="knat")
                    nc.sync.dma_start(out=knt, in_=_sub(knat[m0:m0 + MC, :]))

                    # context: C[m, dv] = sum_n expst[n, m] * v[n, dv]
                    def ctx_group(msb, dc):
                        c_ps = psp.tile([P, 512], F32, tag="mm", name="mm", bufs=2)
                        for nbp in range(NBP):
                            nc.tensor.matmul(
                                c_ps,
                                expst[:, 2 * nbp:2 * nbp + 2,
                                      msb * P:(msb + 1) * P],
                                vall[:, 2 * nbp:2 * nbp + 2,
                                     dc * 512:(dc + 1) * 512],
                                start=(nbp == 0),
                                stop=(nbp == NBP - 1),
                                perf_mode=DR,
                            )
                        return c_ps

                    ot0 = outp.tile([P, D], F32, tag="ostage", name="ostage")
                    c_ps00 = ctx_group(0, 0)

                    recip_row = smallp.tile([1, MC], F32, tag="rrow", name="rrow")
                    nc.vector.reciprocal(recip_row, cs_ps[0:1, :])
                    rp_ps = psp.tile([P, MC // P], F32, tag="csrp", name="rp", bufs=2)
                    for j in range(MC // P):
                        nc.tensor.transpose(
                            rp_ps[:, j:j + 1],
                            recip_row[:, j * P:(j + 1) * P],
                            ident,
                        )
                    recip_pp = smallp.tile([P, MC // P], F32, tag="rpp", name="rpp")
                    nc.vector.tensor_copy(recip_pp, rp_ps)

                    for msb in range(MC // P):
                        r0 = m0 + msb * P
                        ot = ot0 if msb == 0 else outp.tile(
                            [P, D], F32, tag="ostage", name="ostage")
                        for dc in range(D // 512):
                            c_ps = c_ps00 if (msb == 0 and dc == 0) \
                                else ctx_group(msb, dc)
                            nc.vector.scalar_tensor_tensor(
                                out=ot[:, dc * 512:(dc + 1) * 512],
                                in0=c_ps,
                                scalar=recip_pp[:, msb:msb + 1],
                                in1=knt[:, msb, dc * 512:(dc + 1) * 512],
                                op0=mybir.AluOpType.mult,
                                op1=mybir.AluOpType.add,
                            )
                        nc.scalar.dma_start(out=out[r0:r0 + P, :], in_=ot)

                sc = emit_scores(0)
                for mc in range(NMC):
                    sc_cur, sc = sc, (emit_scores(mc + 1)
                                      if mc + 1 < NMC else None)
                    emit_ctx(mc, *sc_cur)

    nc.compile()
    return nc


def _get_nc():
    if "nc" not in _CACHE:
        _CACHE["nc"] = _build()
    return _CACHE["nc"]


def _prep_in_maps(query_input, key_input, Wq, Wk, Wv):
    f8 = ml_dtypes.float8_e4m3
    query_input = np.asarray(query_input, dtype=np.float32)
    key_input = np.asarray(key_input, dtype=np.float32)
    Wq = np.asarray(Wq, dtype=np.float32)
    Wk = np.asarray(Wk, dtype=np.float32)
    Wv = np.asarray(Wv, dtype=np.float32)
    # weight pre-pack: g8 = G.T = (Wq.T @ Wk).T = Wk.T @ Wq, so that
    # kg.T = g8.T @ K.T on device with g8 blocks as the stationary operand
    g8 = np.ascontiguousarray(Wk.T @ Wq).astype(f8)
    wv8 = np.ascontiguousarray(Wv.T).astype(f8)
    in_maps = []
    for b in range(B):
        in_maps.append({
            "x8": np.ascontiguousarray(query_input[b].T).astype(f8),
            "kt8": np.ascontiguousarray(key_input[b].T).astype(f8),
            "knat": np.ascontiguousarray(key_input[b]),
            "g8": g8,
            "wv8": wv8,
        })
    return in_maps


def kernel(query_input, key_input, Wq, Wk, Wv):
    nc = _get_nc()
    in_maps = _prep_in_maps(query_input, key_input, Wq, Wk, Wv)
    res = run_bass_kernel_spmd(nc, in_maps, list(range(B))).results
    return np.stack([res[b]["out"] for b in range(B)], axis=0)
